# revision 3
# baseline (speedup 1.0000x reference)
"""KMeans-HRM graph kernel for 8 Trainium2 NeuronCores (Bass raw blocks).

Math (from the reference):
  S[n,k]   = mask[n,k] * (relu(x[n]@Ww_k) @ Wm_k)      per-node scalar
  b0[n,k]  = x[n] @ Wm_k
  agg[n,k] = sum_{e: dst=n} S[src(e),k]
  hm[n,k]  = mask[n,k]*(b0+agg) > 0
  final    = hm AND (# true heads with k'<k) < 2        (top-2 by index)

disp-A (node-sharded, f32r matmuls + Act relu): computes S^T, b0^T.
disp-B (edge phase): full S in 8 chunk tables [16g+h, 12504] in SBUF;
GPSIMD ap_gather of per-(core,group,substream) dst-sorted src streams;
DVE cumsum (tensor_tensor_scan); boundary ap_gather at per-dst offsets;
PE partition-combine over the 8 groups; shifted subtract + epilogue.
"""
import os
import threading
import time as _time
import numpy as np
from contextlib import ExitStack
from concourse import bass, mybir
from concourse.bass_utils import run_bass_kernel_spmd
from concourse.library_overlay import lower_extended_insts
from concourse import library_config
import bass_rust as _bass_rust

f32 = mybir.dt.float32
f32r = mybir.dt.float32r
i16 = mybir.dt.int16
RELU = mybir.ActivationFunctionType.Relu
COPY = mybir.ActivationFunctionType.Copy
ADD = mybir.AluOpType.add
SUB = mybir.AluOpType.subtract
MULT = mybir.AluOpType.mult
IS_GT = mybir.AluOpType.is_gt
IS_LT = mybir.AluOpType.is_lt

N = 100000
E = 3200000
D = 128
K = 8
NC = 8
SH = N // NC            # 12500 nodes per core
NP = 12544              # padded nodes per core
SUBW = 1568             # dst substream width (8 * 1568 = 12544)
NSUB = 8
CHN = 12500             # src chunk (= group table) size
NE = CHN + 4            # table elems per partition (sentinel at 12500)
LSUB = 6656             # padded substream length (max seen 6539)
WG = 1664               # gather call width (4 per substream)
NCALL = LSUB // WG      # 4
GB = 1600               # boundary gather idxs per substream (1 + 1568 + pad; GB/16 even keeps idx slices 4B-aligned)
SENT = CHN              # sentinel index -> 0.0


def _finalize_ext(nc):
    m = {}
    for lib in library_config.all_libraries:
        for it in lib.instructions:
            m[it] = m.get(it, 0) | (1 << lib.index)
    _bass_rust.insert_library_loads(
        nc, m, len(library_config.all_libraries), library_config.standard.index
    )
    lower_extended_insts(nc)
    return nc


def _build_dispA():
    nc = bass.Bass()
    xT_d = nc.dram_tensor("xT", [D, NP], f32r, kind="ExternalInput")
    mT_d = nc.dram_tensor("mT", [K, NP], f32, kind="ExternalInput")
    ww_d = nc.dram_tensor("ww", [D, K * D], f32r, kind="ExternalInput")
    wme_d = nc.dram_tensor("wme", [D, K * K], f32r, kind="ExternalInput")
    wm8_d = nc.dram_tensor("wm8", [D, K], f32r, kind="ExternalInput")
    sT_d = nc.dram_tensor("sT", [K, NP], f32, kind="ExternalOutput")
    bT_d = nc.dram_tensor("bT", [K, NP], f32, kind="ExternalOutput")

    TIL = 512
    NT = 25                      # 24x512 + 1x256

    def tw(t):
        return TIL if t < 24 else NP - 24 * TIL

    with ExitStack() as es:
        block = es.enter_context(nc.Block())
        ld = es.enter_context(nc.semaphore("ld"))
        pz_s = es.enter_context(nc.semaphore("pz"))    # +1 per Ww matmul
        ar_s = es.enter_context(nc.semaphore("ar"))    # +1 per relu
        ps_s = es.enter_context(nc.semaphore("ps"))    # +1 per Wm-dot matmul
        pb_s = es.enter_context(nc.semaphore("pb"))    # +1 per b0 matmul
        dv_s = es.enter_context(nc.semaphore("dv"))    # +1 per DVE op (2/tile)
        st0_s = es.enter_context(nc.semaphore("st0"))  # stores of even tiles
        st1_s = es.enter_context(nc.semaphore("st1"))  # stores of odd tiles

        xT = es.enter_context(nc.sbuf_tensor("xTs", [D, NP], f32r))
        mT = es.enter_context(nc.sbuf_tensor("mTs", [K, NP], f32))
        ww = es.enter_context(nc.sbuf_tensor("wws", [D, K * D], f32r))
        wme = es.enter_context(nc.sbuf_tensor("wmes", [D, K * K], f32r))
        wm8 = es.enter_context(nc.sbuf_tensor("wm8s", [D, K], f32r))
        wk0 = es.enter_context(nc.sbuf_tensor("wk0", [D, TIL], f32r))
        wk1 = es.enter_context(nc.sbuf_tensor("wk1", [D, TIL], f32r))
        sTt0 = es.enter_context(nc.sbuf_tensor("sTt0", [K, TIL], f32))
        sTt1 = es.enter_context(nc.sbuf_tensor("sTt1", [K, TIL], f32))
        bTt0 = es.enter_context(nc.sbuf_tensor("bTt0", [K, TIL], f32))
        bTt1 = es.enter_context(nc.sbuf_tensor("bTt1", [K, TIL], f32))
        pz0 = es.enter_context(nc.psum_tensor("pz0", [D, TIL], f32))
        pz1 = es.enter_context(nc.psum_tensor("pz1", [D, TIL], f32))
        pS0 = es.enter_context(nc.psum_tensor("pS0", [K, TIL], f32))
        pS1 = es.enter_context(nc.psum_tensor("pS1", [K, TIL], f32))
        pB0 = es.enter_context(nc.psum_tensor("pB0", [K, TIL], f32))
        pB1 = es.enter_context(nc.psum_tensor("pB1", [K, TIL], f32))
        wk = [wk0, wk1]
        sTt = [sTt0, sTt1]
        bTt = [bTt0, bTt1]
        pz = [pz0, pz1]
        pS = [pS0, pS1]
        pB = [pB0, pB1]

        @block.gpsimd
        def _(g):
            g.dma_start(out=xT[:], in_=xT_d[:]).then_inc(ld, 16)
            g.dma_start(out=mT[:], in_=mT_d[:]).then_inc(ld, 16)
            g.dma_start(out=ww[:], in_=ww_d[:]).then_inc(ld, 16)
            g.dma_start(out=wme[:], in_=wme_d[:]).then_inc(ld, 16)
            g.dma_start(out=wm8[:], in_=wm8_d[:]).then_inc(ld, 16)
            sts = [st0_s, st1_s]
            for t in range(NT):
                w = tw(t)
                p = t % 2
                o = t * TIL
                g.wait_ge(dv_s, 2 * t + 1)
                g.dma_start(out=sT_d[:, o : o + w], in_=sTt[p][:, 0:w]).then_inc(
                    sts[p], 16
                )
                g.wait_ge(dv_s, 2 * t + 2)
                g.dma_start(out=bT_d[:, o : o + w], in_=bTt[p][:, 0:w]).then_inc(
                    sts[p], 16
                )
            g.wait_ge(st0_s, 32 * 13)
            g.wait_ge(st1_s, 32 * 12)

        @block.tensor
        def _(pe):
            pe.wait_ge(ld, 80)
            for t in range(NT):
                w = tw(t)
                p = t % 2
                for k in range(K):
                    j = 8 * t + k
                    if j >= 2:
                        pe.wait_ge(ar_s, j - 1)      # pz[j%2] free (relu j-2 done)
                    pe.matmul(
                        pz[j % 2][:, 0:w],
                        ww[:, k * D : (k + 1) * D],
                        xT[:, t * TIL : t * TIL + w],
                        start=True, stop=True,
                    ).then_inc(pz_s, 1)
                    # Wm-dot for head k-1 (so relu k-1 has happened)
                    if k >= 1:
                        jj = 8 * t + k - 1
                        pe.wait_ge(ar_s, jj + 1)
                        if k == 1 and t >= 2:
                            pe.wait_ge(dv_s, 2 * t - 2)   # pS[p] free
                        pe.matmul(
                            pS[p][:, 0:w],
                            wme[:, (k - 1) * K : k * K],
                            wk[jj % 2][:, 0:w],
                            start=(k == 1), stop=False,
                            skip_group_check=True,
                        ).then_inc(ps_s, 1)
                jj = 8 * t + 7
                pe.wait_ge(ar_s, jj + 1)
                pe.matmul(
                    pS[p][:, 0:w],
                    wme[:, 7 * K : 8 * K],
                    wk[jj % 2][:, 0:w],
                    start=False, stop=True,
                    skip_group_check=True,
                ).then_inc(ps_s, 1)
                if t >= 2:
                    pe.wait_ge(dv_s, 2 * t - 1)           # pB[p] free
                pe.matmul(
                    pB[p][:, 0:w],
                    wm8[:],
                    xT[:, t * TIL : t * TIL + w],
                    start=True, stop=True,
                ).then_inc(pb_s, 1)

        @block.scalar
        def _(a):
            for t in range(NT):
                w = tw(t)
                for k in range(K):
                    j = 8 * t + k
                    a.wait_ge(pz_s, j + 1)
                    if j >= 2:
                        a.wait_ge(ps_s, j - 1)       # wk[j%2] free (Wm-dot j-2 done)
                    a.activation(wk[j % 2][:, 0:w], pz[j % 2][:, 0:w], RELU).then_inc(
                        ar_s, 1
                    )

        @block.vector
        def _(v):
            for t in range(NT):
                w = tw(t)
                p = t % 2
                o = t * TIL
                v.wait_ge(ps_s, 8 * (t + 1))
                if t >= 2:
                    # all same-parity stores through tile t-2 complete
                    v.wait_ge([st0_s, st1_s][t % 2], 32 * (t // 2))
                v.tensor_tensor(
                    sTt[p][:, 0:w], pS[p][:, 0:w], mT[:, o : o + w], MULT
                ).then_inc(dv_s, 1)
                v.wait_ge(pb_s, t + 1)
                v.tensor_copy(bTt[p][:, 0:w], pB[p][:, 0:w]).then_inc(dv_s, 1)
    return nc


def _build_dispB():
    nc = bass.Bass()
    S_d = nc.dram_tensor("S", [K, N], f32, kind="ExternalInput")
    bT_d = nc.dram_tensor("bT", [K, NP], f32, kind="ExternalInput")
    mT_d = nc.dram_tensor("mT", [K, NP], f32, kind="ExternalInput")
    idx_d = nc.dram_tensor("idx", [128, NSUB * NCALL * (WG // 16)], i16,
                           kind="ExternalInput")
    bidx_d = nc.dram_tensor("bidx", [128, NSUB * (GB // 16)], i16,
                            kind="ExternalInput")
    sel_d = nc.dram_tensor("sel", [128, K], f32, kind="ExternalInput")
    seln_d = nc.dram_tensor("seln", [128, K], f32, kind="ExternalInput")
    ey8_d = nc.dram_tensor("ey8", [K, K], f32, kind="ExternalInput")
    l8_d = nc.dram_tensor("l8", [K, K], f32, kind="ExternalInput")
    f_d = nc.dram_tensor("f", [K, NP], f32, kind="ExternalOutput")

    IW = WG // 16               # idx cols per gather call
    BW = GB // 16               # bidx cols per substream
    TC = [512, 512, 512, 32]    # combine tile widths (sum = SUBW)

    with ExitStack() as es:
        block = es.enter_context(nc.Block())
        ms = es.enter_context(nc.semaphore("ms"))      # memsets (DVE)
        ld = es.enter_context(nc.semaphore("ld"))      # input loads
        lb0 = es.enter_context(nc.semaphore("lb0"))    # b/m loads, even subs
        lb1 = es.enter_context(nc.semaphore("lb1"))    # b/m loads, odd subs
        gsem = es.enter_context(nc.semaphore("gsem"))  # +1 per main gather
        scs = es.enter_context(nc.semaphore("scs"))    # +1 per scan
        bsem = es.enter_context(nc.semaphore("bsem"))  # +1 per boundary gather
        pes = es.enter_context(nc.semaphore("pes"))    # +1 per combine tile (stop)
        dvh = es.enter_context(nc.semaphore("dvh"))    # +1 per hm tile
        pl8s = es.enter_context(nc.semaphore("pl8s"))  # +1 per l8 matmul
        dvf = es.enter_context(nc.semaphore("dvf"))    # +1 per fin tile
        st = es.enter_context(nc.semaphore("st"))

        chks = es.enter_context(nc.sbuf_tensor("chkss", [128, NE], f32))
        idx = es.enter_context(nc.sbuf_tensor("idxs", [128, NSUB * NCALL * IW], i16))
        bidx = es.enter_context(nc.sbuf_tensor("bidxs", [128, NSUB * BW], i16))
        gt0 = es.enter_context(nc.sbuf_tensor("gt0", [128, WG], f32))
        gt1 = es.enter_context(nc.sbuf_tensor("gt1", [128, WG], f32))
        C = es.enter_context(nc.sbuf_tensor("C", [128, LSUB + 1], f32))
        zb = es.enter_context(nc.sbuf_tensor("zb", [128, WG], f32))
        G0 = es.enter_context(nc.sbuf_tensor("G0", [128, GB], f32))
        G1 = es.enter_context(nc.sbuf_tensor("G1", [128, GB], f32))
        sel = es.enter_context(nc.sbuf_tensor("sels", [128, K], f32))
        seln = es.enter_context(nc.sbuf_tensor("selns", [128, K], f32))
        ey8 = es.enter_context(nc.sbuf_tensor("ey8s", [K, K], f32))
        l8 = es.enter_context(nc.sbuf_tensor("l8s", [K, K], f32))
        bt0 = es.enter_context(nc.sbuf_tensor("bt0", [K, SUBW], f32))
        bt1 = es.enter_context(nc.sbuf_tensor("bt1", [K, SUBW], f32))
        mt0 = es.enter_context(nc.sbuf_tensor("mt0", [K, SUBW], f32))
        mt1 = es.enter_context(nc.sbuf_tensor("mt1", [K, SUBW], f32))
        hm0 = es.enter_context(nc.sbuf_tensor("hm0", [K, 512], f32))
        hm1 = es.enter_context(nc.sbuf_tensor("hm1", [K, 512], f32))
        hm2 = es.enter_context(nc.sbuf_tensor("hm2", [K, 512], f32))
        hm3 = es.enter_context(nc.sbuf_tensor("hm3", [K, 512], f32))
        fout = es.enter_context(nc.sbuf_tensor("fouts", [K, NP], f32))
        pc0 = es.enter_context(nc.psum_tensor("pc0", [K, 512], f32))
        pc1 = es.enter_context(nc.psum_tensor("pc1", [K, 512], f32))
        pc2 = es.enter_context(nc.psum_tensor("pc2", [K, 512], f32))
        pc3 = es.enter_context(nc.psum_tensor("pc3", [K, 512], f32))
        pl0 = es.enter_context(nc.psum_tensor("pl0", [K, 512], f32))
        pl1 = es.enter_context(nc.psum_tensor("pl1", [K, 512], f32))
        gt = [gt0, gt1]
        G = [G0, G1]
        bt = [bt0, bt1]
        mt = [mt0, mt1]
        hm = [hm0, hm1, hm2, hm3]
        pc = [pc0, pc1, pc2, pc3]
        pl = [pl0, pl1]

        @block.gpsimd
        def _(g):
            g.dma_start(out=idx[:], in_=idx_d[:]).then_inc(ld, 16)
            g.dma_start(out=bidx[:], in_=bidx_d[:]).then_inc(ld, 16)
            g.wait_ge(ms, 1)
            for gg in range(8):
                g.dma_start(
                    out=chks[16 * gg : 16 * gg + 8, 0:CHN],
                    in_=S_d[:, CHN * gg : CHN * (gg + 1)],
                ).then_inc(ld, 16)
            g.dma_start(out=sel[:], in_=sel_d[:]).then_inc(ld, 16)
            g.dma_start(out=seln[:], in_=seln_d[:]).then_inc(ld, 16)
            g.dma_start(out=ey8[:], in_=ey8_d[:]).then_inc(ld, 16)
            g.dma_start(out=l8[:], in_=l8_d[:]).then_inc(ld, 16)
            lbs = [lb0, lb1]
            for sub in range(2):
                g.dma_start(
                    out=bt[sub % 2][:], in_=bT_d[:, SUBW * sub : SUBW * (sub + 1)]
                ).then_inc(lbs[sub % 2], 16)
                g.dma_start(
                    out=mt[sub % 2][:], in_=mT_d[:, SUBW * sub : SUBW * (sub + 1)]
                ).then_inc(lbs[sub % 2], 16)
            g.wait_ge(ld, 16 * 14)     # 8 chks + sel + seln + ey8 + l8 + idx + bidx
            g.wait_ge(ms, 3)
            _serial = bool(os.environ.get("BASS_B_SERIAL"))
            for sub in range(NSUB):
                for c in range(NCALL):
                    j = NCALL * sub + c
                    if _serial:
                        g.wait_ge(scs, j)
                    elif j >= 2:
                        g.wait_ge(scs, j - 1)         # gt[j%2] free
                    g.ap_gather(
                        gt[j % 2][:],
                        chks[:],
                        idx[:, j * IW : (j + 1) * IW],
                        channels=128, num_elems=NE, d=1, num_idxs=WG,
                    ).then_inc(gsem, 1)
                # boundary gather for this substream after its 4 scans
                g.wait_ge(scs, NCALL * (sub + 1))
                if sub >= 2:
                    g.wait_ge(pes, 4 * (sub - 1))     # G[sub%2] free
                g.ap_gather(
                    G[sub % 2][:],
                    C[:],
                    bidx[:, sub * BW : (sub + 1) * BW],
                    channels=128, num_elems=LSUB + 1, d=1, num_idxs=GB,
                ).then_inc(bsem, 1)
                if 1 <= sub < NSUB - 1:
                    nx = sub + 1
                    g.wait_ge(dvh, 4 * sub)         # bt/mt[nx%2] consumers done
                    g.dma_start(
                        out=bt[nx % 2][:], in_=bT_d[:, SUBW * nx : SUBW * (nx + 1)]
                    ).then_inc(lbs[nx % 2], 16)
                    g.dma_start(
                        out=mt[nx % 2][:], in_=mT_d[:, SUBW * nx : SUBW * (nx + 1)]
                    ).then_inc(lbs[nx % 2], 16)
            g.wait_ge(dvf, 4 * NSUB)
            g.dma_start(out=f_d[:], in_=fout[:]).then_inc(st, 16)
            g.wait_ge(st, 16)

        @block.vector
        def _(v):
            v.memset(chks[:], 0.0).then_inc(ms, 1)
            v.memset(zb[:], 0.0).then_inc(ms, 1)
            v.memset(C[:, 0:1], 0.0).then_inc(ms, 1)

            def scans(sub):
                for c in range(NCALL):
                    j = NCALL * sub + c
                    v.wait_ge(gsem, j + 1)
                    if c == 0 and sub >= 1:
                        v.wait_ge(bsem, sub)          # C free (bnd of sub-1 done)
                    else:
                        # drain: chained initial reads scan j-1's last column
                        v.wait_ge(scs, j)
                    v.tensor_tensor_scan(
                        C[:, 1 + WG * c : 1 + WG * (c + 1)],
                        gt[j % 2][:],
                        zb[:],
                        0.0 if c == 0 else C[:, WG * c : WG * c + 1],
                        ADD, ADD,
                    ).then_inc(scs, 1)

            def epi(sub):
                v.wait_ge([lb0, lb1][sub % 2], 32 * (sub // 2 + 1))
                for j in range(4):
                    w = TC[j]
                    off = 512 * j
                    i = 4 * sub + j
                    v.wait_ge(pes, i + 1)
                    if i >= 4:
                        v.wait_ge(pl8s, i - 3)        # hm[i%4] free (l8 i-4 done)
                        v.wait_ge(dvf, i - 3)         # ... and STT2 i-4 done
                    v.scalar_tensor_tensor(
                        hm[i % 4][:, 0:w], pc[i % 4][:, 0:w], 0.0,
                        mt[sub % 2][:, off : off + w],
                        IS_GT, MULT,
                    ).then_inc(dvh, 1)
                for j in range(4):
                    w = TC[j]
                    off = 512 * j
                    i = 4 * sub + j
                    v.wait_ge(pl8s, i + 1)
                    v.scalar_tensor_tensor(
                        fout[:, SUBW * sub + off : SUBW * sub + off + w],
                        pl[i % 2][:, 0:w], 2.0, hm[i % 4][:, 0:w],
                        IS_LT, MULT,
                    ).then_inc(dvf, 1)

            scans(0)
            for sub in range(1, NSUB):
                scans(sub)
                epi(sub - 1)
            epi(NSUB - 1)

        @block.tensor
        def _(pe):
            pe.wait_ge(ld, 16 * 14)

            def combine(sub):
                pe.wait_ge(bsem, sub + 1)
                pe.wait_ge([lb0, lb1][sub % 2], 32 * (sub // 2 + 1))
                for j in range(4):
                    i = 4 * sub + j
                    if i >= 4:
                        pe.wait_ge(dvh, i - 3)        # pc[i%4] free (STT1 i-4 done)
                    off = 512 * j
                    w = TC[j]
                    pe.matmul(
                        pc[i % 4][:, 0:w], sel[:],
                        G[sub % 2][:, 1 + off : 1 + off + w],
                        start=True, stop=False, skip_group_check=True,
                    )
                    pe.matmul(
                        pc[i % 4][:, 0:w], seln[:],
                        G[sub % 2][:, off : off + w],
                        start=False, stop=False, skip_group_check=True,
                    )
                    pe.matmul(
                        pc[i % 4][:, 0:w], ey8[:],
                        bt[sub % 2][:, off : off + w],
                        start=False, stop=True, skip_group_check=True,
                    ).then_inc(pes, 1)

            def l8mm(sub):
                for j in range(4):
                    i = 4 * sub + j
                    w = TC[j]
                    pe.wait_ge(dvh, i + 1)
                    if i >= 2:
                        pe.wait_ge(dvf, i - 1)        # pl[i%2] free
                    pe.matmul(
                        pl[i % 2][:, 0:w],
                        l8[:],
                        hm[i % 4][:, 0:w],
                        start=True, stop=True,
                    ).then_inc(pl8s, 1)

            combine(0)
            for sub in range(1, NSUB):
                combine(sub)
                l8mm(sub - 1)
            l8mm(NSUB - 1)
    return nc


def _edge_prep(edge_index):
    src = edge_index[0].astype(np.int32)
    dst = edge_index[1].astype(np.int32)
    core = dst // SH
    dl = dst - core * SH
    g = src // CHN
    so = (src - g * CHN).astype(np.int16)
    key = (core * 8 + g) * SH + dl
    order = np.argsort(key)   # unstable ok: only dst-grouping matters
    so_s = so[order]
    q = dl // SUBW
    cgq = (core * 8 + g)[order] * NSUB + q[order]
    cnt_cgq = np.bincount(cgq, minlength=NC * 8 * NSUB)
    assert cnt_cgq.max() <= LSUB, cnt_cgq.max()
    starts = np.zeros(NC * 8 * NSUB, np.int64)
    starts[1:] = np.cumsum(cnt_cgq)[:-1]
    streams = np.full((NC * 8 * NSUB, LSUB), SENT, np.int16)
    pos = np.arange(len(so_s)) - starts[cgq]
    streams[cgq, pos] = so_s
    # wrap main idx: [NC,8,NSUB,NCALL,WG] -> [NC, 128, NSUB*NCALL*IW]
    IW = WG // 16
    sw = streams.reshape(NC, 8, NSUB * NCALL, IW, 16)
    idx_w = np.ascontiguousarray(sw.transpose(0, 1, 4, 2, 3)).reshape(
        NC, 128, NSUB * NCALL * IW
    )
    # boundary positions
    cnt_cgd = np.bincount(key, minlength=NC * 8 * SH).reshape(NC, 8, SH)
    cnt_pad = np.zeros((NC, 8, NP), np.int64)
    cnt_pad[:, :, :SH] = cnt_cgd
    bpos = np.cumsum(
        cnt_pad.reshape(NC, 8, NSUB, SUBW), axis=3
    ).astype(np.int16)                               # values in [0, LSUB]
    bfull = np.zeros((NC, 8, NSUB, GB), np.int16)    # col 0 = 0 (C[:,0] = 0)
    bfull[:, :, :, 1 : 1 + SUBW] = bpos
    BW = GB // 16
    bw = bfull.reshape(NC, 8, NSUB, BW, 16)
    bidx_w = np.ascontiguousarray(bw.transpose(0, 1, 4, 2, 3)).reshape(
        NC, 128, NSUB * BW
    )
    return idx_w, bidx_w


def kernel(x, edge_index, mask, Ww, Wm):
    x = np.asarray(x, dtype=np.float32)
    edge_index = np.asarray(edge_index)
    mask = np.asarray(mask, dtype=np.float32)
    Ww = np.asarray(Ww, dtype=np.float32)
    Wm = np.asarray(Wm, dtype=np.float32)

    prep = {}

    def _prep_thread():
        prep["idx"], prep["bidx"] = _edge_prep(edge_index)

    th = threading.Thread(target=_prep_thread)
    th.start()
    _tm0 = _time.time()

    # disp-A inputs
    wwt = np.ascontiguousarray(Ww.transpose(1, 0, 2)).reshape(D, K * D)
    wme = np.zeros((D, K * K), np.float32)
    for k in range(K):
        wme[:, k * K + k] = Wm[k, :, 0]
    wm8 = np.ascontiguousarray(Wm[:, :, 0].T)        # [D, K]
    xr = x.reshape(NC, SH, D)
    mr = mask.reshape(NC, SH, K)
    mapsA = []
    for c in range(NC):
        xTp = np.zeros((D, NP), np.float32)
        xTp[:, :SH] = xr[c].T
        mTp = np.zeros((K, NP), np.float32)
        mTp[:, :SH] = mr[c].T
        mapsA.append({"xT": xTp, "mT": mTp, "ww": wwt, "wme": wme, "wm8": wm8})

    _tr = bool(os.environ.get("BASS_KERNEL_TRACE"))
    _t0 = _time.time()
    if os.environ.get("BASS_KERNEL_TIMES"):
        print(f"[kern] mapsA {_t0-_tm0:.2f}s", flush=True)
    ncA = _build_dispA()
    _t1 = _time.time()
    resA = run_bass_kernel_spmd(ncA, mapsA, list(range(NC)), trace=_tr)
    rA = resA.results
    _t2 = _time.time()
    if os.environ.get("BASS_KERNEL_TIMES"):
        print(f"[kern] buildA {_t1-_t0:.2f}s dispA {_t2-_t1:.2f}s "
              f"hwA={resA.exec_time_ns}", flush=True)

    S_full = np.empty((K, N), np.float32)
    for c in range(NC):
        S_full[:, c * SH : (c + 1) * SH] = rA[c]["sT"][:, :SH]

    th.join()
    selt = np.zeros((128, K), np.float32)
    for g in range(8):
        for h in range(K):
            selt[16 * g + h, h] = 1.0
    l8m = np.triu(np.ones((K, K), np.float32), 1)    # l8[kp, m] = 1 iff kp < m

    ey8 = np.eye(K, dtype=np.float32)
    mapsB = []
    for c in range(NC):
        mapsB.append(
            {
                "S": S_full,
                "bT": rA[c]["bT"],
                "mT": mapsA[c]["mT"],
                "idx": prep["idx"][c],
                "bidx": prep["bidx"][c],
                "sel": selt,
                "seln": -selt,
                "ey8": ey8,
                "l8": l8m,
            }
        )
    _t3 = _time.time()
    ncB = _finalize_ext(_build_dispB())
    _t4 = _time.time()
    resB = run_bass_kernel_spmd(ncB, mapsB, list(range(NC)), trace=_tr)
    rB = resB.results
    _t5 = _time.time()
    if os.environ.get("BASS_KERNEL_TIMES"):
        print(f"[kern] glue {_t3-_t2:.2f}s buildB {_t4-_t3:.2f}s "
              f"dispB {_t5-_t4:.2f}s hwB={resB.exec_time_ns}", flush=True)

    out = np.empty((N, K), np.float32)
    for c in range(NC):
        out[c * SH : (c + 1) * SH] = rB[c]["f"][:, :SH].T
    if os.environ.get("BASS_KERNEL_TIMES"):
        print(f"[kern] assemble {_time.time()-_t5:.2f}s", flush=True)
    return out
